# revision 3
# baseline (speedup 1.0000x reference)
"""Trainium2 Bass kernel v2 for the attention-LSTM model (B=512,F=8,S=256,H=512,T=64).

Sharding: data-parallel over batch B across 8 cores (64 rows each).
v2 changes vs v1:
  - Encoder: gates PSUM in half-tiles [64,2,4,128] (bufs=3), weights column-
    permuted host-side to (h-chunk, gate, 128) order so tails process h-chunks
    as they complete; elementwise tail split across ACT/DVE/Pool; hT/h_bf
    double-buffered -> PE streams with no per-step bubble.
  - Decoder: single ACT table set (sigmoid/tanh/square/copy) the whole loop:
    softmax exp computed as sigma/(1-sigma), BN rstd via Newton rsqrt on DVE.
    Einsum waves sized to 1 PSUM bank (4 batch rows each) with b-index remap
    so consolidation is one DMA per 8 rows; BN stats from the transposed ctx
    via ACT/DVE accumulators into the AllReduce payload [128,10]; one readback
    DMA; LSTM h-part matmuls issued before the collective so they execute
    under its latency; LSTM tails chunk-pipelined like the encoder.
"""

import numpy as np

NCORES = 8
B, F, S, H, T = 512, 8, 256, 512, 64
BS = B // NCORES  # 64
G4 = 4 * H  # 2048
EPS = 1e-5

_CACHE = {}


def _build():
    import concourse.bass as bass
    import concourse.tile as tile
    from concourse import mybir
    from concourse.masks import make_identity

    FP = mybir.dt.float32
    BF = mybir.dt.bfloat16
    I32 = mybir.dt.int32
    AF = mybir.ActivationFunctionType
    ALU = mybir.AluOpType
    RG = [list(range(NCORES))]

    nc = bass.Bass(num_devices=NCORES)

    # ---------------- DRAM parameters (per-core shards; host preps layouts) ---
    def din(name, shape):
        return nc.declare_dram_parameter(name, list(shape), FP, isOutput=False)

    d_x1t = din("x1t", [F, BS, S])        # x1 shard transposed (f, b, s)
    d_x2b = din("x2b", [BS, T])           # x2 shard natural (b, t)
    d_x2row = din("x2row", [1, T * BS])   # x2^T flat
    d_wxT = din("wxT", [F, G4])           # Wih_e^T, cols permuted (c,j,128)
    d_bih_e = din("bih_e", [1, G4])       # permuted
    d_bhh_e = din("bhh_e", [1, G4])       # permuted
    d_whhT = din("whhT", [H, G4])         # Whh_e^T, cols permuted
    d_bn_e_g = din("bn_e_g", [F, 1])
    d_bn_e_b = din("bn_e_b", [F, 1])
    d_attnT_i = din("attnT_i", [2, S])    # attn_W^T rows 0:2 (feature-swapped)
    d_attnT_h = din("attnT_h", [H, S])    # attn_W^T rows 2:514
    d_attn_b = din("attn_b", [1, S])
    d_combT_i = din("combT_i", [2, H])    # comb_W^T rows 0:2 (feature-swapped)
    d_combT_h = din("combT_h", [H, H])    # comb_W^T rows 2:514
    d_comb_b = din("comb_b", [128, 4])    # comb bias per-partition chunks
    d_bn_d_g5 = din("bn_d_g5", [128, 5])  # cols0-3 ctx chunks, col4 row1 = y
    d_bn_d_b5 = din("bn_d_b5", [128, 5])
    d_bn_d_gi = din("bn_d_gi", [2, 1])    # rows (x2, y) inp-feature gammas
    d_bn_d_bi = din("bn_d_bi", [2, 1])
    d_wihdT = din("wihdT", [H, G4])       # cols permuted
    d_whhdT = din("whhdT", [H, G4])       # cols permuted
    d_bih_d = din("bih_d", [1, G4])       # permuted
    d_bhh_d = din("bhh_d", [1, G4])       # permuted
    d_outw2 = din("outw2", [128, 8])      # out_W chunks interleaved w/ zeros
    d_outb = din("outb", [1, BS])         # out_b replicated
    d_out = nc.declare_dram_parameter("out", [T, BS], FP, isOutput=True)

    with tile.TileContext(nc) as tc, __import__("contextlib").ExitStack() as ctx:
        singles = ctx.enter_context(tc.tile_pool(name="singles", bufs=1))
        acts = ctx.enter_context(tc.tile_pool(name="acts", bufs=1))
        dves = ctx.enter_context(tc.tile_pool(name="dves", bufs=2))
        one1 = ctx.enter_context(tc.tile_pool(name="one1", bufs=1))
        dram = ctx.enter_context(tc.tile_pool(name="dram", bufs=1, space="DRAM"))
        dram2 = ctx.enter_context(tc.tile_pool(name="dram2", bufs=2, space="DRAM"))
        decw = ctx.enter_context(tc.tile_pool(name="decw", bufs=1))
        encwcm = tc.tile_pool(name="encw", bufs=1)
        encw = encwcm.__enter__()
        prolcm = tc.tile_pool(name="prol", bufs=1)
        prol = prolcm.__enter__()
        ldpcm = tc.tile_pool(name="ldp", bufs=2)
        ldp = ldpcm.__enter__()
        # encoder-phase psum for prologue matmuls (freed before decoder)
        psecm = tc.tile_pool(name="pse", bufs=2, space="PSUM")
        pse = psecm.__enter__()

        # ---------------- constants -----------------------------------------
        ident_bf = singles.tile([128, 128], BF)
        make_identity(nc, ident_bf)
        ident_fp = prol.tile([128, 128], FP, tag="ident_fp")
        make_identity(nc, ident_fp)
        ones64 = singles.tile([BS, 1], FP)
        nc.vector.memset(ones64, 1.0)
        ones1f = singles.tile([1, 1], FP)
        nc.vector.memset(ones1f, 1.0)
        onesrow_b = singles.tile([1, BS], BF)
        nc.vector.memset(onesrow_b, 1.0)
        e01b = singles.tile([1, 2], BF)
        nc.vector.memset(e01b, 0.0)
        nc.vector.memset(e01b[0:1, 1:2], 1.0)
        e10b = singles.tile([1, 2], BF)
        nc.vector.memset(e10b, 0.0)
        nc.vector.memset(e10b[0:1, 0:1], 1.0)
        eps128 = singles.tile([128, 1], FP)
        nc.vector.memset(eps128, EPS)
        eps1 = singles.tile([1, 1], FP)
        nc.vector.memset(eps1, EPS)
        shift1 = singles.tile([128, 1], I32)
        nc.vector.memset(shift1, 1)
        magic5 = singles.tile([128, 5], I32)
        nc.vector.memset(magic5, 0x5F3759DF)

        # ---------------- persistent decoder weights (load first) -----------
        def ld_bf(dst, dsrc):
            t = ldp.tile([128, G4], FP, tag="ldtmp")
            tv = t[: dsrc.shape[0], : dsrc.shape[1]]
            nc.sync.dma_start(tv, dsrc[:])
            nc.vector.tensor_copy(dst, tv)

        wihd_bf = decw.tile([128, 4, G4], BF, tag="wihd_bf")
        whhd_bf = decw.tile([128, 4, G4], BF, tag="whhd_bf")
        for c in range(4):
            ld_bf(wihd_bf[:, c], d_wihdT[c * 128 : (c + 1) * 128, :])
            ld_bf(whhd_bf[:, c], d_whhdT[c * 128 : (c + 1) * 128, :])
        bd_row = decw.tile([1, G4], BF)
        t1 = ldp.tile([128, G4], FP, tag="ldtmp")
        nc.sync.dma_start(t1[0:1, :], d_bih_d[:])
        t2 = ldp.tile([128, G4], FP, tag="ldtmp")
        nc.sync.dma_start(t2[0:1, :], d_bhh_d[:])
        nc.vector.tensor_add(bd_row, t1[0:1, :], t2[0:1, :])
        attnh_bf = decw.tile([128, 4, S], BF, tag="attnh_bf")
        for c in range(4):
            ld_bf(attnh_bf[:, c], d_attnT_h[c * 128 : (c + 1) * 128, :])
        attni_bf = decw.tile([2, S], BF)
        ld_bf(attni_bf, d_attnT_i)
        attnb_bf = decw.tile([1, S], BF)
        ld_bf(attnb_bf, d_attn_b)
        combh_bf = decw.tile([128, 4, H], BF, tag="combh_bf")
        for c in range(4):
            ld_bf(combh_bf[:, c], d_combT_h[c * 128 : (c + 1) * 128, :])
        combi_bf = decw.tile([2, H], BF)
        ld_bf(combi_bf, d_combT_i)
        comb_bp = singles.tile([128, 4], FP)
        nc.sync.dma_start(comb_bp, d_comb_b[:])
        bn_d_g5 = singles.tile([128, 5], FP)
        nc.sync.dma_start(bn_d_g5, d_bn_d_g5[:])
        bn_d_b5 = singles.tile([128, 5], FP)
        nc.sync.dma_start(bn_d_b5, d_bn_d_b5[:])
        bn_d_gi = singles.tile([2, 1], FP)
        nc.sync.dma_start(bn_d_gi, d_bn_d_gi[:])
        bn_d_bi = singles.tile([2, 1], FP)
        nc.sync.dma_start(bn_d_bi, d_bn_d_bi[:])
        outw2 = decw.tile([128, 8], BF)
        ld_bf(outw2, d_outw2)
        outb_row = decw.tile([1, BS], BF)
        ld_bf(outb_row, d_outb)
        x2row = decw.tile([1, T * BS], BF)  # x2^T flat (x2 feature of inp)
        for cq in range(4):
            x2c = prol.tile([1, T * BS // 4], FP, tag="x2c")
            nc.sync.dma_start(x2c, d_x2row[0:1, cq * 1024 : (cq + 1) * 1024])
            nc.vector.tensor_copy(x2row[0:1, cq * 1024 : (cq + 1) * 1024], x2c)

        # ---------------- x1 load/cast + BN stats ----------------------------
        x1t_bf = encw.tile([F + 1, BS, S], BF)
        nc.vector.memset(x1t_bf[0:1], 1.0)
        xsum = prol.tile([F, 4], FP)
        xsq = prol.tile([F, 4], FP)
        for cq in range(4):
            x1q = prol.tile([F, BS // 4, S], FP, tag="x1q")
            nc.sync.dma_start(x1q, d_x1t[:, cq * 16 : (cq + 1) * 16, :])
            nc.gpsimd.dma_start(
                out=x1t_bf[1 : F + 1, cq * 16 : (cq + 1) * 16, :],
                in_=d_x1t[:, cq * 16 : (cq + 1) * 16, :],
            )
            nc.scalar.activation(x1q, x1q, AF.Copy, accum_out=xsum[:, cq : cq + 1])
            nc.scalar.activation(x1q, x1q, AF.Square, accum_out=xsq[:, cq : cq + 1])
        xs2 = prol.tile([F, 2], FP)
        nc.vector.tensor_add(xs2[:, 0:1], xsum[:, 0:1], xsum[:, 1:2])
        nc.vector.tensor_add(xs2[:, 1:2], xsum[:, 2:3], xsum[:, 3:4])
        xstat = prol.tile([F, 2], FP)
        nc.vector.tensor_add(xstat[:, 0:1], xs2[:, 0:1], xs2[:, 1:2])
        nc.vector.tensor_add(xs2[:, 0:1], xsq[:, 0:1], xsq[:, 1:2])
        nc.vector.tensor_add(xs2[:, 1:2], xsq[:, 2:3], xsq[:, 3:4])
        nc.vector.tensor_add(xstat[:, 1:2], xs2[:, 0:1], xs2[:, 1:2])

        # x2 column stats
        x2b_sb = prol.tile([BS, T], FP)
        nc.sync.dma_start(x2b_sb, d_x2b[:])
        ps_x2 = pse.tile([1, 2, T], FP, tag="pse")
        nc.tensor.matmul(ps_x2[:, 0], ones64, x2b_sb, start=True, stop=True)
        x2sq = prol.tile([BS, T], FP)
        nc.scalar.activation(x2sq, x2b_sb, AF.Square)
        nc.tensor.matmul(ps_x2[:, 1], ones64, x2sq, start=True, stop=True)

        # one-time AllReduce payload [1, 160]
        pay0 = prol.tile([1, 160], FP)
        nc.vector.memset(pay0, 0.0)
        ps_ft = pse.tile([2, F], FP, tag="pse")
        nc.tensor.transpose(ps_ft, xstat, ident_fp[:F, :F])
        fst2 = prol.tile([2, F], FP, tag="fst2")
        nc.vector.tensor_copy(fst2, ps_ft)
        nc.scalar.copy(pay0[0:1, 16 : 16 + T], ps_x2[:, 0])
        nc.scalar.copy(pay0[0:1, 16 + T : 16 + 2 * T], ps_x2[:, 1])
        ar0_in = dram.tile([1, 160], FP)
        ar0_out = dram.tile([1, 160], FP)
        nc.sync.dma_start(ar0_in, pay0)
        nc.sync.dma_start(ar0_in[0:1, 0:F], fst2[0:1, :])
        nc.sync.dma_start(ar0_in[0:1, F : 2 * F], fst2[1:2, :])
        nc.gpsimd.collective_compute(
            "AllReduce", ALU.add, replica_groups=RG,
            ins=[ar0_in[:].opt()], outs=[ar0_out[:].opt()],
        )
        fstat = prol.tile([F, 2], FP)
        _a0 = ar0_out[0:1, 0:1]
        nc.sync.dma_start(
            fstat,
            bass.AP(tensor=_a0.tensor, offset=_a0.offset, ap=[[1, F], [F, 2]]),
        )
        x2stat = prol.tile([1, 2, T], FP)
        nc.sync.dma_start(x2stat[:, 0], ar0_out[0:1, 16 : 16 + T])
        nc.sync.dma_start(x2stat[:, 1], ar0_out[0:1, 16 + T : 16 + 2 * T])

        # exact global x2-col sums scaled by 1/ncores: re-summed by the
        # per-step AllReduce to recover the exact global stats
        px2 = singles.tile([1, 2, T], FP)
        nc.scalar.mul(px2, x2stat, 1.0 / NCORES)

        # encoder BN fold
        NBS = float(B * S)
        bn_e_g = prol.tile([F, 1], FP)
        nc.sync.dma_start(bn_e_g, d_bn_e_g[:])
        bn_e_b = prol.tile([F, 1], FP)
        nc.sync.dma_start(bn_e_b, d_bn_e_b[:])
        mf = prol.tile([F, 1], FP)
        nc.scalar.mul(mf, fstat[:, 0:1], 1.0 / NBS)
        vf = prol.tile([F, 1], FP)
        m2f = prol.tile([F, 1], FP)
        nc.scalar.activation(m2f, mf, AF.Square)
        nc.scalar.mul(vf, fstat[:, 1:2], 1.0 / NBS)
        nc.vector.tensor_sub(vf, vf, m2f)
        nc.scalar.activation(vf, vf, AF.Sqrt, bias=eps128[:F])
        nc.vector.reciprocal(vf, vf)
        af = prol.tile([F, 1], FP)
        nc.vector.tensor_mul(af, bn_e_g, vf)
        cf = prol.tile([F, 1], FP)
        nc.vector.tensor_mul(cf, mf, af)
        nc.vector.tensor_sub(cf, bn_e_b, cf)

        wxT_f = prol.tile([F, G4], FP)
        nc.sync.dma_start(wxT_f, d_wxT[:])
        wx_bf = encw.tile([F + 1, G4], BF)
        wxs_bf = prol.tile([F, G4], BF, tag="wxs_bf")
        nc.vector.tensor_scalar_mul(wxs_bf, wxT_f, af)
        nc.sync.dma_start(wx_bf[1 : F + 1, :], wxs_bf)
        ps_b1 = pse.tile([1, 2, 512], FP, tag="pse")
        ps_b2 = pse.tile([1, 2, 512], FP, tag="pse")
        for j in range(2):
            nc.tensor.matmul(
                ps_b1[:, j], cf, wxT_f[:, j * 512 : (j + 1) * 512],
                start=True, stop=True,
            )
            nc.tensor.matmul(
                ps_b2[:, j], cf, wxT_f[:, (2 + j) * 512 : (3 + j) * 512],
                start=True, stop=True,
            )
        t1 = ldp.tile([128, G4], FP, tag="ldtmp")
        nc.sync.dma_start(t1[0:1, :], d_bih_e[:])
        t2 = ldp.tile([128, G4], FP, tag="ldtmp")
        nc.sync.dma_start(t2[0:1, :], d_bhh_e[:])
        bias_acc = prol.tile([1, G4], FP, tag="bias_acc")
        nc.vector.tensor_add(bias_acc, t1[0:1, :], t2[0:1, :])
        bias_bf = prol.tile([1, G4], BF, tag="bias_bf")
        nc.vector.tensor_add(
            bias_bf[0:1, 0:1024],
            ps_b1.rearrange("p a b -> p (a b)"), bias_acc[0:1, 0:1024],
        )
        nc.vector.tensor_add(
            bias_bf[0:1, 1024:2048],
            ps_b2.rearrange("p a b -> p (a b)"), bias_acc[0:1, 1024:2048],
        )
        nc.sync.dma_start(wx_bf[0:1, :], bias_bf)

        whh_bf = encw.tile([128, 4, G4], BF, tag="whh_bf")
        for c in range(4):
            ld_bf(whh_bf[:, c], d_whhT[c * 128 : (c + 1) * 128, :])

        # ---------------- state ----------------------------------------------
        # hT/h_bf double-buffered across steps
        hT = [singles.tile([128, 4, BS], BF, tag=f"hT{i}", name=f"hT{i}") for i in range(2)]
        nc.vector.memset(hT[0], 0.0)
        nc.vector.memset(hT[1], 0.0)
        h_bf = [singles.tile([BS, H], BF, tag=f"hbf{i}", name=f"hbf{i}") for i in range(2)]
        c_sb = singles.tile([BS, H], FP)
        nc.vector.memset(c_sb, 0.0)
        inp2 = singles.tile([2, BS], BF)   # row0 = x2_t, row1 = prev y
        nc.vector.memset(inp2, 0.0)
        y_st = singles.tile([2, BS], FP)   # row1 = y fp32
        nc.vector.memset(y_st, 0.0)
        enc_dram = dram.tile([S, BS, H], BF)

        ldpcm.__exit__(None, None, None)
        prolcm.__exit__(None, None, None)
        psecm.__exit__(None, None, None)
        # PSUM pools: gates halves (2 banks x2) + ps_t (1 bank); decoder adds
        # psA (1 bank x2) + psW (1 bank) -> total 8 banks.
        psg = ctx.enter_context(tc.tile_pool(name="psg", bufs=2, space="PSUM"))
        pst = ctx.enter_context(tc.tile_pool(name="pst", bufs=1, space="PSUM"))

        # ================= ENCODER LOOP ======================================
        # gates layout: [64, cc(2), j(4), 128] per half; global chunk c=half*2+cc
        def lstm_tail(g_half, half, cur, nxt, whichT, th=None):
            """Elementwise LSTM tail for one half (2 chunks). Gate order in
            permuted layout: j in (i,f,g,o). Produces h chunks + transposes."""
            for cc in range(2):
                c = half * 2 + cc
                sl = g_half[:, cc]  # [64, 4, 128] psum
                si = one1.tile([BS, 4, 128], BF, tag="si")
                nc.scalar.activation(si[:, 0], sl[:, 0], AF.Sigmoid)
                nc.scalar.activation(si[:, 1], sl[:, 1], AF.Sigmoid)
                tg = one1.tile([BS, 128], BF, tag="tg")
                nc.scalar.activation(tg, sl[:, 2], AF.Tanh)
                nc.scalar.activation(si[:, 3], sl[:, 3], AF.Sigmoid)
                csl = c_sb[:, c * 128 : (c + 1) * 128]
                m1 = one1.tile([BS, 128], FP, tag=f"m1_{cc}")
                nc.gpsimd.tensor_mul(m1, si[:, 1], csl)
                m2 = one1.tile([BS, 128], FP, tag=f"m2_{cc}")
                nc.vector.tensor_mul(m2, si[:, 0], tg)
                nc.vector.tensor_add(csl, m1, m2)
                tc_ = one1.tile([BS, 128], BF, tag=f"tc_{cc}")
                nc.scalar.activation(tc_, csl, AF.Tanh)
                hsl = h_bf[cur][:, c * 128 : (c + 1) * 128]
                nc.vector.tensor_mul(hsl, si[:, 3], tc_)
                ps_t = pst.tile([128, BS], BF, tag="ps_t")
                nc.tensor.transpose(ps_t, hsl, ident_bf[:BS, :BS])
                nc.vector.tensor_copy(hT[whichT][:, c], ps_t)
                if th is not None:
                    nc.scalar.activation(th[:, c], ps_t, AF.Tanh)

        for s in range(S):
            cur, prv = s % 2, (s + 1) % 2
            for half in range(2):
                g = psg.tile([BS, 2, 4, 128], FP, tag="gates")
                for cc in range(2):
                    off = (half * 2 + cc) * 512
                    nc.tensor.matmul(
                        g[:, cc], x1t_bf[:, :, s], wx_bf[:, off : off + 512],
                        start=True, stop=False,
                    )
                for k in range(4):
                    for cc in range(2):
                        off = (half * 2 + cc) * 512
                        nc.tensor.matmul(
                            g[:, cc], hT[prv][:, k],
                            whh_bf[:, k, off : off + 512],
                            start=False, stop=(k == 3),
                        )
                lstm_tail(g, half, cur, prv, cur)
            nc.sync.dma_start(enc_dram[s], h_bf[cur])

        hT_fin = (S - 1) % 2  # hT buffer holding the final encoder state
        encwcm.__exit__(None, None, None)
        encsp = ctx.enter_context(tc.tile_pool(name="encsp", bufs=1))
        enc_sb0 = encsp.tile([128, BS, H], BF, tag="enc_sb0")
        enc_sb1 = encsp.tile([128, BS, H], BF, tag="enc_sb1")
        nc.sync.dma_start(enc_sb0, enc_dram[0:128])
        nc.sync.dma_start(enc_sb1, enc_dram[128:256])
        enc_sb = [enc_sb0, enc_sb1]

        # decoder-phase psum pools: psA 2x1 bank, psW 1 bank; psg/pst reused
        psA = ctx.enter_context(tc.tile_pool(name="psA", bufs=2, space="PSUM"))
        psW = ctx.enter_context(tc.tile_pool(name="psW", bufs=1, space="PSUM"))
        scatp = ctx.enter_context(tc.tile_pool(name="scatp", bufs=2))

        th = singles.tile([128, 4, BS], BF, tag="th")
        nc.scalar.activation(th, hT[hT_fin], AF.Tanh)
        wT_sb = singles.tile([128, 2, BS], BF, tag="wT_sb")
        apiN = singles.tile([128, 4, BS], BF, tag="apiN")
        ctxc = singles.tile([BS, H], BF, tag="ctxc")
        pay = singles.tile([128, 10], FP, tag="pay")
        nc.vector.memset(pay, 0.0)
        rb = singles.tile([128, 10], FP, tag="rb")
        junk1 = singles.tile([128, BS], BF, tag="junk1")
        junk2 = singles.tile([128, BS], FP, tag="junk2")

        # pre-loop: inp2 row0 = x2[:, 0]; row1 (y) = 0 from memset
        nc.sync.dma_start(inp2[0:1, :], x2row[0:1, 0:BS])

        # ================= DECODER LOOP ======================================
        for k in range(T):
            cur, prv = k % 2, (k + 1) % 2
            whichT = (hT_fin + k) % 2      # hT buffer carrying h_k
            newT = (hT_fin + k + 1) % 2    # buffer for h_{k+1}
            # prefetch exp table while logits matmuls run
            dmy = acts.tile([1, 1], FP, tag="dmy")
            nc.scalar.activation(dmy, eps1, AF.Exp)
            # ---- attention logits [64, 256] ----
            ps_l = psA.tile([BS, S], FP, tag="psA")
            nc.tensor.matmul(ps_l, onesrow_b, attnb_bf, start=True, stop=False)
            for c in range(4):
                nc.tensor.matmul(
                    ps_l, hT[whichT][:, c], attnh_bf[:, c],
                    start=False, stop=False,
                )
            nc.tensor.matmul(ps_l, inp2, attni_bf, start=False, stop=True)
            # ---- softmax (exp table already resident) ----
            w_f = acts.tile([BS, S], FP, tag="w_f")
            zs = acts.tile([BS, 1], FP, tag="zs")
            nc.scalar.activation(w_f, ps_l, AF.Exp, accum_out=zs)
            rz = acts.tile([BS, 1], FP, tag="rz")
            nc.vector.reciprocal(rz, zs)
            wn_bf = acts.tile([BS, S], BF, tag="wn_bf")
            nc.vector.tensor_scalar_mul(wn_bf, w_f, rz)
            ps_w = psW.tile([128, 2, BS], BF, tag="ps_w")
            for sc in range(2):
                nc.tensor.transpose(
                    ps_w[:, sc], wn_bf[:, sc * 128 : (sc + 1) * 128],
                    ident_bf[:BS, :BS],
                )
            nc.vector.tensor_copy(wT_sb, ps_w)
            # ---- einsum: 16 waves of 4 b; b = wp*8 + j*2 + q ----
            for wp in range(8):
                scat = scatp.tile([128, 2, H], BF, tag="scat")
                for q in range(2):
                    ps_e = psA.tile([128, H], FP, tag="psA")
                    for j in range(4):
                        b = wp * 8 + j * 2 + q
                        for sc in range(2):
                            nc.tensor.matmul(
                                ps_e[32 * j : 32 * j + 1, :],
                                wT_sb[:, sc, b : b + 1],
                                enc_sb[sc][:, b, :],
                                start=(sc == 0), stop=(sc == 1),
                                tile_position=(0, 32 * j),
                            )
                    if q == 0:
                        nc.scalar.copy(scat[:, q], ps_e)
                    else:
                        nc.vector.tensor_copy(scat[:, q], ps_e)
                nc.sync.dma_start(
                    ctxc[wp * 8 : (wp + 1) * 8, :], scat[0:128:32, :, :]
                )
                if wp == 4:
                    nc.scalar.activation(dmy, eps1, AF.Sigmoid)
            # gates tiles + dep-free bias matmuls fill the consolidation wait
            gtiles = []
            for half in range(2):
                g = psg.tile([BS, 2, 4, 128], FP, tag="gates")
                gtiles.append(g)
                for cc in range(2):
                    off = (half * 2 + cc) * 512
                    nc.tensor.matmul(
                        g[:, cc], onesrow_b, bd_row[:, off : off + 512],
                        start=True, stop=False,
                    )
            # ---- transpose ctx -> ps_ct [128, 4, 64]; stats into pay ----
            ps_ct = psA.tile([128, 4, BS], BF, tag="psA")
            for c in range(4):
                nc.tensor.transpose(
                    ps_ct[:, c], ctxc[:, c * 128 : (c + 1) * 128],
                    ident_bf[:BS, :BS],
                )
            for c in range(4):
                nc.scalar.activation(
                    junk1, ps_ct[:, c], AF.Copy, accum_out=pay[:, c : c + 1]
                )
                nc.scalar.activation(
                    junk2, ps_ct[:, c], AF.Square, accum_out=pay[:, 5 + c : 6 + c]
                )
            # inp-feature stats: rows (x2, y) into col 4 / col 9; x2 row then
            # overwritten with the exact precomputed global share
            nc.scalar.activation(
                junk2[0:2, :], y_st, AF.Copy, accum_out=pay[0:2, 4:5],
            )
            nc.scalar.activation(
                junk2[0:2, :], y_st, AF.Square, accum_out=pay[0:2, 9:10],
            )
            nc.vector.tensor_copy(pay[0:1, 4:5], px2[0:1, 0, k : k + 1])
            nc.vector.tensor_copy(pay[0:1, 9:10], px2[0:1, 1, k : k + 1])
            ar_in = dram2.tile([128, 10], FP, tag="ar_in")
            ar_out = dram2.tile([128, 10], FP, tag="ar_out")
            nc.sync.dma_start(ar_in, pay)
            # ---- LSTM gate h-parts: run during the collective ----
            for half in range(2):
                g = gtiles[half]
                for kk in range(4):
                    for cc in range(2):
                        off = (half * 2 + cc) * 512
                        nc.tensor.matmul(
                            g[:, cc], hT[whichT][:, kk],
                            whhd_bf[:, kk, off : off + 512],
                            start=False, stop=False,
                        )
            nc.gpsimd.collective_compute(
                "AllReduce", ALU.add, replica_groups=RG,
                ins=[ar_in[:].opt()], outs=[ar_out[:].opt()],
            )
            nc.sync.dma_start(rb, ar_out[:])
            # ---- affine from stats: [128, 5] grid (cols 0-3 ctx, col4 y) ----
            mean5 = dves.tile([128, 5], FP, tag="mean5")
            nc.scalar.mul(mean5, rb[:, 0:5], 1.0 / B)
            m2 = dves.tile([128, 5], FP, tag="m2t")
            nc.vector.tensor_mul(m2, mean5, mean5)
            q5 = dves.tile([128, 5], FP, tag="q5")
            nc.scalar.mul(q5, rb[:, 5:10], 1.0 / B)
            veps = dves.tile([128, 5], FP, tag="veps")
            nc.vector.scalar_tensor_tensor(
                veps, q5, EPS, m2, ALU.add, ALU.subtract
            )
            # Newton rsqrt: y0 via bit trick, 2 iterations
            vi = veps.bitcast(I32)
            ti = dves.tile([128, 5], I32, tag="ti")
            nc.vector.tensor_scalar(ti, vi, shift1, None, ALU.logical_shift_right)
            yi = dves.tile([128, 5], I32, tag="yi")
            nc.vector.tensor_tensor(yi, magic5, ti, ALU.subtract)
            yf = yi.bitcast(FP)
            u1 = dves.tile([128, 5], FP, tag="u1")
            u2 = dves.tile([128, 5], FP, tag="u2")
            for _ in range(2):
                nc.vector.tensor_mul(u1, yf, yf)
                nc.vector.tensor_mul(u2, u1, veps)
                nc.vector.tensor_scalar(u2, u2, -0.5, 1.5, ALU.mult, ALU.add)
                nc.vector.tensor_mul(yf, yf, u2)
            a5 = dves.tile([128, 5], FP, tag="a5")
            nc.vector.tensor_mul(a5, bn_d_g5, yf)
            c5 = dves.tile([128, 5], FP, tag="c5")
            nc.vector.tensor_mul(c5, mean5, a5)
            nc.vector.tensor_sub(c5, bn_d_b5, c5)
            # ---- apiN + inpN ----
            for c in range(4):
                nc.vector.tensor_scalar(
                    apiN[:, c], ps_ct[:, c], a5[:, c : c + 1], c5[:, c : c + 1],
                    ALU.mult, ALU.add,
                )
            inpN = acts.tile([2, BS], BF, tag="inpN")
            nc.vector.tensor_scalar(
                inpN, inp2, a5[0:2, 4:5], c5[0:2, 4:5], ALU.mult, ALU.add
            )
            # ---- comb matmul -> xcT (sigmoid + bias fused) ----
            ps_x = psA.tile([128, 4, BS], FP, tag="psA")
            for jc in range(4):
                for kc in range(4):
                    nc.tensor.matmul(
                        ps_x[:, jc],
                        combh_bf[:, kc, jc * 128 : (jc + 1) * 128],
                        apiN[:, kc],
                        start=(kc == 0), stop=False,
                    )
                nc.tensor.matmul(
                    ps_x[:, jc], combi_bf[:, jc * 128 : (jc + 1) * 128], inpN,
                    start=False, stop=True,
                )
            xcT = acts.tile([128, 4, BS], BF, tag="xcT")
            for jc in range(4):
                nc.scalar.activation(
                    xcT[:, jc], ps_x[:, jc], AF.Sigmoid,
                    bias=comb_bp[:, jc : jc + 1],
                )
            # ---- LSTM gate xc-parts + tails ----
            for half in range(2):
                g = gtiles[half]
                for kk in range(4):
                    for cc in range(2):
                        off = (half * 2 + cc) * 512
                        nc.tensor.matmul(
                            g[:, cc], xcT[:, kk],
                            wihd_bf[:, kk, off : off + 512],
                            start=False, stop=(kk == 3),
                        )
                lstm_tail(g, half, cur, prv, newT, th=th)
            # ---- y = tanh(h) @ out_W^T + out_b; row0 = x2_{k+1} ----
            k2 = (k + 1) % T
            ps_y = psA.tile([2, BS], FP, tag="psA")
            nc.tensor.matmul(ps_y, e01b, outb_row, start=True, stop=False)
            for c in range(4):
                nc.tensor.matmul(
                    ps_y, outw2[:, 2 * c : 2 * c + 2], th[:, c],
                    start=False, stop=False,
                )
            nc.tensor.matmul(
                ps_y, e10b, x2row[0:1, k2 * BS : (k2 + 1) * BS],
                start=False, stop=True,
            )
            nc.scalar.copy(y_st, ps_y)
            nc.vector.tensor_copy(inp2, y_st)
            nc.sync.dma_start(d_out[k : k + 1, :], y_st[1:2, :])

    # ---- post-pass: this walrus build accepts at most ONE sync-wait per
    # instruction; hoist extra waits onto injected NoOps on the same engine.
    nfix = 0
    for fn in nc.m.functions:
        for blk in fn.blocks:
            newlist = []
            for ins in blk.instructions:
                si = ins.sync_info
                if si is not None and si.on_wait and len(si.on_wait) > 1:
                    waits = list(si.on_wait)
                    for w in waits[:-1]:
                        nop = mybir.InstNoOp(name=f"WH-{nfix}")
                        nfix += 1
                        nop.engine = ins.engine
                        nop.sync_info = mybir.SyncInfo(on_wait=[w], on_update=[])
                        newlist.append(nop)
                    ins.sync_info = mybir.SyncInfo(
                        on_wait=[waits[-1]], on_update=list(si.on_update or [])
                    )
                newlist.append(ins)
            blk.instructions[:] = newlist

    return nc


def _get_nc():
    if "nc" not in _CACHE:
        _CACHE["nc"] = _build()
    return _CACHE["nc"]


def _outw2(out_W):
    w = np.zeros((128, 8), np.float32)
    ch = out_W.reshape(4, 128)
    for c in range(4):
        w[:, 2 * c + 1] = ch[c]
    return w


def _permute_cols(m):
    """[rows, 2048] gate-major (j,512) -> (c,j,128) order."""
    r = m.reshape(m.shape[0], 4, 4, 128)      # [rows, j, c, 128]
    return np.ascontiguousarray(
        r.transpose(0, 2, 1, 3).reshape(m.shape[0], 2048)
    )


def _prep_inputs(inputs):
    """Full inputs -> list of 8 per-core input dicts (layout prep only)."""
    f32 = np.float32
    x1 = np.asarray(inputs["x1"], f32)
    x2 = np.asarray(inputs["x2"], f32)
    attn_WT = np.ascontiguousarray(np.asarray(inputs["attn_W"], f32).T)  # [514, 256]
    comb_WT = np.ascontiguousarray(np.asarray(inputs["comb_W"], f32).T)  # [514, 512]
    out_W = np.asarray(inputs["out_W"], f32)  # [1, 512]
    bn_d_g = np.asarray(inputs["bn_d_g"], f32)
    bn_d_b = np.asarray(inputs["bn_d_b"], f32)
    g5 = np.zeros((128, 5), f32)
    b5 = np.zeros((128, 5), f32)
    g5[:, 0:4] = bn_d_g[2:].reshape(4, 128).T
    b5[:, 0:4] = bn_d_b[2:].reshape(4, 128).T
    g5[0, 4] = bn_d_g[1]   # x2 feature (reference feature 1) at row 0
    b5[0, 4] = bn_d_b[1]
    g5[1, 4] = bn_d_g[0]   # y feature (reference feature 0) at row 1
    b5[1, 4] = bn_d_b[0]
    shared = {
        "wxT": _permute_cols(np.ascontiguousarray(np.asarray(inputs["Wih_e"], f32).T)),
        "bih_e": _permute_cols(np.asarray(inputs["bih_e"], f32).reshape(1, -1)),
        "bhh_e": _permute_cols(np.asarray(inputs["bhh_e"], f32).reshape(1, -1)),
        "whhT": _permute_cols(np.ascontiguousarray(np.asarray(inputs["Whh_e"], f32).T)),
        "bn_e_g": np.asarray(inputs["bn_e_g"], f32).reshape(-1, 1),
        "bn_e_b": np.asarray(inputs["bn_e_b"], f32).reshape(-1, 1),
        "attnT_i": np.ascontiguousarray(attn_WT[0:2][::-1]),
        "attnT_h": np.ascontiguousarray(attn_WT[2:]),
        "attn_b": np.asarray(inputs["attn_b"], f32).reshape(1, -1),
        "combT_i": np.ascontiguousarray(comb_WT[0:2][::-1]),
        "combT_h": np.ascontiguousarray(comb_WT[2:]),
        "comb_b": np.ascontiguousarray(
            np.asarray(inputs["comb_b"], f32).reshape(4, 128).T
        ),
        "bn_d_g5": g5,
        "bn_d_b5": b5,
        "bn_d_gi": np.ascontiguousarray(bn_d_g[:2][::-1].reshape(2, 1)),
        "bn_d_bi": np.ascontiguousarray(bn_d_b[:2][::-1].reshape(2, 1)),
        "wihdT": _permute_cols(np.ascontiguousarray(np.asarray(inputs["Wih_d"], f32).T)),
        "whhdT": _permute_cols(np.ascontiguousarray(np.asarray(inputs["Whh_d"], f32).T)),
        "bih_d": _permute_cols(np.asarray(inputs["bih_d"], f32).reshape(1, -1)),
        "bhh_d": _permute_cols(np.asarray(inputs["bhh_d"], f32).reshape(1, -1)),
        "outw2": _outw2(out_W),
        "outb": np.full((1, BS), float(np.asarray(inputs["out_b"], f32)[0]), f32),
    }
    in_maps = []
    for i in range(NCORES):
        x1s = x1[i * BS : (i + 1) * BS]  # [64, 8, 256]
        x2s = x2[i * BS : (i + 1) * BS]  # [64, 64]
        m = dict(shared)
        m["x1t"] = np.ascontiguousarray(x1s.transpose(1, 0, 2))
        m["x2b"] = np.ascontiguousarray(x2s)
        m["x2row"] = np.ascontiguousarray(x2s.T).reshape(1, -1)
        in_maps.append(m)
    return in_maps


def run(inputs, trace=False):
    from concourse.bass_utils import run_bass_kernel_spmd

    nc = _get_nc()
    in_maps = _prep_inputs(inputs)
    res = run_bass_kernel_spmd(
        nc, in_maps, core_ids=list(range(NCORES)), trace=trace
    )
    outs = [r["out"] for r in res.results]  # each [T, BS]
    full = np.concatenate([o.T for o in outs], axis=0)  # [B, T]
    return full, res


def kernel(**inputs):
    full, _ = run(inputs, trace=False)
    return full
